# revision 4
# baseline (speedup 1.0000x reference)
"""Trainium2 Bass kernel for the leaky-ReLU arccos covariance-grid conv1d problem.

Computation (see problem reference):
  k: (B,B,N,T,2) f32.  k_gp = k[...,0], k_ntk = k[...,1]
  v[b,t] = k_gp[b,b,0,t];  std = sqrt(max(v,0)) padded with N-1 zeros
  std_x[b0,t] = std[b0,t];  std_y[b1,n,t] = std[b1,n+t]
  rho = clip(k_gp / max(std_x*std_y, EPS), +-RHO_LIM)
  With leak a (graded a=1): one_m=(1-a)^2=0, coef=1+a^2=2 =>
    c0 = std_x*std_y*rho  = min(k_gp, RHO_LIM*std_x*std_y)   (k_gp >= 0)
  kg = conv1d(c0, w, pad 1) + beta;  kn = conv1d(k_ntk, w, pad 1) + kg
  out = stack([kg, kn], -1)

Sharding: b0 (leading batch axis) across 8 cores; each core handles the
(8,128,1024,2) slice k[b0] independently.

Performance notes (measured on this part):
 - The 2e-2 harness tolerance admits bf16; the exact rounding chain used
   here measures ~7e-3 max rel err.  All I/O is bf16: 4.1 MiB in + 4.0
   MiB out per core vs 20 MiB for the f32 baseline.
 - DMA here is DESCRIPTOR-rate limited (~20-25ns/descriptor per queue
   regardless of size) until descriptors reach ~8 KiB, so all host
   layouts are partition-major: each SBUF partition's data is contiguous
   in DRAM, giving 4-16 KiB descriptors (vs 2 KiB rows).
 - The (N, B, T) Hankel table std_y[b1, n+t] is expanded on-device by ONE
   SBUF->SBUF DMA: windows are expanded at the native row spacing SDP
   (not the consumed width TP), which makes every partition's whole table
   row a single contiguous run of the flat std row: sqh[p, col] =
   flat[p + col].  128 descriptors of 18.7 KiB, no HBM traffic.
 - Compute is DVE-only (7 tensor_tensor ops per b1 tile, all-bf16 packed
   operands for the DVE fast mode) + 2 ACT copies folding the conv tap
   scale and beta.  The PE identity-matmul conv used by an earlier
   version runs at the low pstate (~625ns per 512-row matmul) and loses
   to two extra DVE adds.

Per-core device program, per b1 tile of (N=128 partitions, T=1024):
  DVE:  m = sxm*sqh; c0 = min(gp, m); t1 = c0[j]+c0[j+1]; t2 = t1+c0[j+2]
        u1 = ntk[j]+ntk[j+1]; u2 = u1+ntk[j+2]; s = u2+t2 (f32)
  ACT:  kg = Copy(t2*wl + beta); kn = Copy(s*wl + beta) -> out accum tile
  DMA:  qSP: input chunks + middle store; qACT: tables/Hankel + stores.
"""

import numpy as np
import ml_dtypes
from contextlib import ExitStack

import concourse.tile as tile
from concourse import bacc, mybir
from concourse.alu_op_type import AluOpType
from concourse.bass_utils import run_bass_kernel_spmd
from bass_rust import AP as RawAP

B, N, T = 8, 128, 1024
TP = T + 2              # padded tile width
SDP = 1154              # padded std row spacing (even, >= N + TP - 1 = 1153)
SQW = (B - 1) * SDP + TP        # 9104: consumed sqh width
SFL = SQW + N - 1               # 9231: flat spd row length (partition over-read)
EPS = 1e-12
RHO_LIM = 1.0 - 1e-6
F32 = mybir.dt.float32
BF16 = mybir.dt.bfloat16
NPBF = ml_dtypes.bfloat16

_prog_cache = {}


def _build_program(wl, beta):
    """One SPMD program, identical on all 8 cores (data differs per core).

    Equal conv taps wl and leak a=1 are assumed (the fast path guards on
    this); beta is folded into the two ACT copies.
    """
    nc = bacc.Bacc(
        "TRN2",
        target_bir_lowering=False,
        debug=False,
        enable_asserts=False,
        num_devices=8,
    )
    x_d = nc.dram_tensor("x", [2, N, B * TP], BF16, kind="ExternalInput").ap()
    spd_d = nc.dram_tensor("spd", [1, SFL], BF16, kind="ExternalInput").ap()
    sxm_d = nc.dram_tensor("sxm", [1, TP], BF16, kind="ExternalInput").ap()
    out_d = nc.dram_tensor("out", [N, B * 2 * T], BF16, kind="ExternalOutput").ap()

    with tile.TileContext(nc) as tc, ExitStack() as ctx:
        const = ctx.enter_context(tc.tile_pool(name="const", bufs=1))

        spd_sb = const.tile([1, SFL], BF16)
        sxr_sb = const.tile([1, TP], BF16)
        sqh_sb = const.tile([N, SQW], BF16)
        sxm_sb = const.tile([N, TP], BF16)
        xg_sb = const.tile([N, B * TP], BF16)
        xn_sb = const.tile([N, B * TP], BF16)
        out_sb = const.tile([N, B * 2 * T], BF16)

        # qACT: tables first.  The Hankel expansion reads overlapping
        # windows of the single-partition flat std row: partition p of
        # sqh_sb gets flat[p : p + SQW], so sqh_sb[p, b1*SDP + j] =
        # spd[b1, p + j] = std_y[b1, p, j-1].  One descriptor/partition.
        nc.scalar.dma_start(spd_sb[:], spd_d)
        sv = spd_sb[:]
        c0w = 2 * SDP + TP      # cover b1 in {0,1,2} before the input bulk
        nc.scalar.dma_start(
            sqh_sb[:, 0:c0w], RawAP(sv.tensor, sv.offset, [[SFL, 1], [1, N], [1, c0w]])
        )
        nc.scalar.dma_start(sxr_sb[:], sxm_d)
        rv = sxr_sb[:]
        nc.scalar.dma_start(
            sxm_sb[:], RawAP(rv.tensor, rv.offset, [[TP, 1], [0, N], [1, TP]])
        )
        nc.scalar.dma_start(
            sqh_sb[:, c0w:SQW],
            RawAP(sv.tensor, sv.offset + c0w, [[SFL, 1], [1, N], [1, SQW - c0w]]),
        )

        # qSP: input planes in two chunks per channel (first 2 b1 tiles,
        # then the remaining 6) — 4.1/12.3 KiB descriptors.
        cut = 2 * TP
        nc.sync.dma_start(xg_sb[:, 0:cut], x_d[0, :, 0:cut])
        nc.sync.dma_start(xn_sb[:, 0:cut], x_d[1, :, 0:cut])
        nc.sync.dma_start(xg_sb[:, cut:], x_d[0, :, cut:])
        nc.sync.dma_start(xn_sb[:, cut:], x_d[1, :, cut:])

        # persistent DVE work tiles (DVE is serial; reuse is free)
        m_t = const.tile([N, TP], BF16)
        c0p = const.tile([N, TP], BF16)
        t1_t = const.tile([N, T], BF16)
        t2_t = const.tile([N, T], BF16)
        u1_t = const.tile([N, T], BF16)
        u2_t = const.tile([N, T], BF16)
        s_t = const.tile([N, T], F32)

        for b1 in range(B):
            gp = xg_sb[:, b1 * TP : (b1 + 1) * TP]
            ntk = xn_sb[:, b1 * TP : (b1 + 1) * TP]
            sq = sqh_sb[:, b1 * SDP : b1 * SDP + TP]
            og = out_sb[:, b1 * 2 * T : b1 * 2 * T + T]
            on = out_sb[:, b1 * 2 * T + T : (b1 + 1) * 2 * T]

            nc.vector.tensor_tensor(m_t[:], sxm_sb[:], sq, op=AluOpType.mult)
            nc.vector.tensor_tensor(c0p[:], gp, m_t[:], op=AluOpType.min)
            nc.vector.tensor_tensor(
                t1_t[:], c0p[:, 0:T], c0p[:, 1 : T + 1], op=AluOpType.add
            )
            nc.vector.tensor_tensor(
                t2_t[:], t1_t[:], c0p[:, 2:TP], op=AluOpType.add
            )
            nc.scalar.activation(
                og, t2_t[:], mybir.ActivationFunctionType.Copy,
                bias=beta, scale=wl,
            )
            nc.vector.tensor_tensor(
                u1_t[:], ntk[:, 0:T], ntk[:, 1 : T + 1], op=AluOpType.add
            )
            nc.vector.tensor_tensor(
                u2_t[:], u1_t[:], ntk[:, 2:TP], op=AluOpType.add
            )
            nc.vector.tensor_tensor(s_t[:], u2_t[:], t2_t[:], op=AluOpType.add)
            nc.scalar.activation(
                on, s_t[:], mybir.ActivationFunctionType.Copy,
                bias=beta, scale=wl,
            )

            # chunked stores: b1 0-2 and 6-7 on qACT, 3-5 on qSP (which is
            # idle after the input loads) — 6-12 KiB descriptors.
            if b1 == 2:
                nc.scalar.dma_start(
                    out_d[:, 0 : 6 * T], out_sb[:, 0 : 6 * T]
                )
            elif b1 == 5:
                nc.sync.dma_start(
                    out_d[:, 6 * T : 12 * T], out_sb[:, 6 * T : 12 * T]
                )
            elif b1 == 7:
                nc.scalar.dma_start(
                    out_d[:, 12 * T : 16 * T], out_sb[:, 12 * T : 16 * T]
                )

    nc.compile()
    return nc


def _host_reference(k, leak, alpha, beta):
    """Numpy fallback replicating the reference exactly (any leak/alpha)."""
    k_gp, k_ntk = k[..., 0], k[..., 1]
    Bb, _, Nn, Tt = k_gp.shape
    ar = np.arange(Bb)
    v = k_gp[ar, ar, 0, :]
    v_pad = np.pad(v, ((0, 0), (0, Nn - 1)))
    std = np.sqrt(np.maximum(v_pad, 0.0))
    std_x = std[:, :Tt][:, None, None, :]
    std_y = np.lib.stride_tricks.sliding_window_view(std, Tt, axis=1)[None]
    denom = np.maximum(std_x * std_y, EPS)
    rho = np.clip(k_gp / denom, -RHO_LIM, RHO_LIM).astype(np.float32)
    a = max(float(leak), 0.0)
    theta = np.arccos(rho)
    s = np.sqrt(1.0 - rho * rho)
    one_m = (1.0 - a) ** 2
    coef = 1.0 + a * a
    sxy = (std_x * std_y).astype(np.float32)
    c0 = sxy / (2 * np.pi) * (one_m * s + rho * (coef * np.pi - one_m * theta))
    c1 = (coef * np.pi - one_m * theta) / (2 * np.pi)
    w = np.maximum(np.asarray(alpha, np.float32).reshape(-1), 0.0)

    def conv(x):
        xp = np.pad(x, ((0, 0), (0, 0), (0, 0), (1, 1)))
        return (
            w[0] * xp[..., :Tt] + w[1] * xp[..., 1 : Tt + 1] + w[2] * xp[..., 2 : Tt + 2]
        ).astype(np.float32)

    b = max(float(beta), 0.0)
    kg = conv(c0.astype(np.float32)) + b
    kn = conv((c1 * k_ntk).astype(np.float32)) + (kg - b) + b
    return np.stack([kg, kn], axis=-1).astype(np.float32)


def kernel(k, leak, alpha, beta, _want_profile=False):
    k = np.asarray(k, dtype=np.float32)
    a = max(float(np.asarray(leak)), 0.0)
    w = np.maximum(np.asarray(alpha, dtype=np.float32).reshape(-1), np.float32(0.0))
    b_eff = max(float(np.asarray(beta)), 0.0)

    fast = (
        k.shape == (B, B, N, T, 2)
        and (a == 1.0)
        and w.shape[0] == 3
        and w[0] == w[1] == w[2]
        and k.min() >= 0.0
    )
    if not fast:
        return _host_reference(k, leak, alpha, beta)

    wl = float(w[0])
    key = (wl, b_eff)
    if key not in _prog_cache:
        _prog_cache[key] = _build_program(wl, b_eff)
    nc = _prog_cache[key]

    # host prep: bf16 partition-major padded inputs + tiny std tables
    kb = k.astype(NPBF)                                  # (B,B,N,T,2)
    x = np.zeros((B, 2, N, B, TP), dtype=NPBF)
    x[:, 0, :, :, 1 : T + 1] = kb[..., 0].transpose(0, 2, 1, 3)
    x[:, 1, :, :, 1 : T + 1] = kb[..., 1].transpose(0, 2, 1, 3)
    x = x.reshape(B, 2, N, B * TP)

    ar = np.arange(B)
    v = k[ar, ar, 0, :, 0]                               # (B, T) f32
    v_pad = np.pad(v, ((0, 0), (0, N - 1)))              # (B, T+N-1 = 1151)
    std = np.sqrt(np.maximum(v_pad, 0.0)).astype(np.float32)
    # flat shifted std rows at SDP spacing: spd[b1*SDP + u] = std[b1, u-1]
    spd = np.zeros((B, SDP), dtype=np.float32)
    spd[:, 0] = std[:, 0]                                # u=0 -> t=-1 pad, any >=0
    spd[:, 1 : T + N] = std
    flat = spd.reshape(-1)[:SFL].reshape(1, SFL).astype(NPBF)

    rl = np.float32(RHO_LIM)
    sxm_all = np.zeros((B, 1, TP), dtype=np.float32)
    sxm_all[:, 0, 1 : T + 1] = rl * std[:, :T]
    sxm_all[:, 0, 0] = rl * std[:, 0]
    sxm_all[:, 0, T + 1] = rl * std[:, T - 1]
    sxm_all = sxm_all.astype(NPBF)

    in_maps = [
        {"x": x[c], "spd": flat, "sxm": sxm_all[c]} for c in range(B)
    ]

    res = run_bass_kernel_spmd(
        nc, in_maps, core_ids=list(range(8)), trace=_want_profile
    )
    out = np.empty((B, B, N, T, 2), dtype=np.float32)
    for c, r in enumerate(res.results):
        # (N, B, 2, T) -> (B, N, T, 2)
        out[c] = (
            r["out"].reshape(N, B, 2, T).transpose(1, 0, 3, 2).astype(np.float32)
        )
    if _want_profile:
        kernel.last_exec_time_ns = res.exec_time_ns
        kernel.last_results = res
    return out


kernel.last_exec_time_ns = None
kernel.last_results = None
